# revision 15
# baseline (speedup 1.0000x reference)
"""CrossEfficientAttention on 8 Trainium2 NeuronCores — fp8 DoubleRow version.

Batch-parallel sharding: n=8 batch items, one per core (no collectives).

Per-core math (item x_q, x_k, x_v : [256, 6400]):
    q  = Wq x_q + bq ; k = Wk x_k (bk cancels over the l-softmax) ; v = Wv x_v
    k_sm = softmax_l(k); q_sm = softmax_ch/head(q)
    ctx  = k_sm @ v^T (per head); out = Wr @ (ctx^T @ q_sm) + br' + x_q
  with br' = br + Wr @ bv (the bv term reduces to a constant because
  sum_head q_sm == 1), so bv never appears on the device.

Numerics: the attention term is a few % of the output (the residual
dominates), so the whole attention path runs in fp8e4m3 on the PE with
DoubleRow perf mode (256-deep contraction in one matmul, ~2x bf16 rate);
exp() outputs are scaled by 1/4 (bias -ln4) to stay inside fp8 range —
the scale cancels exactly in both softmax normalizations.  The residual
is bf16; output ships bf16.

Structure (one fused streaming loop + tiny boundary + output pass):
  Pass 1 per 256-l unit: k/v projected via one DoubleRow matmul per
    128-l block into [l, ch] psum, exp(k) on ACT -> fp8 ksmT, v copied
    on DVE -> fp8 vT (4-D layout with a ones column per 64-ch group);
    Gram ctx accumulates via 2 fused [128,130] DoubleRow matmuls per
    unit — the ones column yields S_k for free (cross-group halves of
    each 130-wide output are garbage and never read).  Interleaved q
    pipeline: DoubleRow q-projection, exp on ACT, head-sums via one
    DoubleRow matmul with a 0/1 block matrix, fast reciprocal, PE
    broadcast, normalize in place on GPSIMD.
  Boundary: 1/S_k scale of the per-head diag blocks, 2 PE transposes,
    at = ctxn^T . Wr^T via 4 small matmuls, cast to fp8.
  Pass 2: po = at^T . q_sm (one DoubleRow matmul per 512 cols), then
    out = po + br' + x_q via one scalar_tensor_tensor -> bf16 -> DMA.
"""

from contextlib import ExitStack

import ml_dtypes
import numpy as np

import concourse.bacc as bacc
import concourse.bass as bass
import concourse.tile as tile
from concourse import mybir
from concourse.bass_utils import run_bass_kernel_spmd

F32 = mybir.dt.float32
BF16 = mybir.dt.bfloat16
FP8 = mybir.dt.float8e4
EXP = mybir.ActivationFunctionType.Exp
DR = mybir.MatmulPerfMode.DoubleRow
MULT = mybir.AluOpType.mult
ADD = mybir.AluOpType.add

N_CORES = 8
N, CIN, H_IMG, W_IMG = 8, 256, 80, 80
L = H_IMG * W_IMG            # 6400
HEADS = 8
LW = 512                     # streaming l tile width
NLW = (L + LW - 1) // LW     # 13 (12x512 + 1x256)
NU = L // 256                # 25 gram units of 256 l

LN4 = float(np.log(4.0))

# c8 fp8 const tile [128, 2, 776]: per k-subtile: wk(256)|wv(256)|wq(256)|bones(8)
C8_INNER = 784
CF_COLS = 133                # f32: bql [128,2] | brp [128,2] | kbias | ident


def _emit(tc: tile.TileContext, ins: dict, out_ap: bass.AP):
    nc = tc.nc
    es = ExitStack()

    # ---------------- persistent consts (4 DMAs) ----------------
    cpool = es.enter_context(tc.tile_pool(name="consts", bufs=1))
    c8 = cpool.tile([128, 2, C8_INNER], FP8, name="c8")
    ind8 = cpool.tile([8, 256], BF16, name="ind8")
    wrt4 = cpool.tile([64, 4, 256], BF16, name="wrt4")
    cf = cpool.tile([128, CF_COLS], F32, name="cf")
    WK8 = c8[:, :, 0:256]
    WV8 = c8[:, :, 256:512]
    WQ8 = c8[:, :, 512:768]
    BONES8 = c8[:, :, 768:776]
    BQL = cf[:, 0:2]
    BRP = cf[:, 2:4]
    KBIAS = cf[:, 4:5]
    IDENT = cf[:, 5:133]

    at8 = cpool.tile([128, 2, 256], FP8, name="at8")
    eq8 = cpool.tile([128, 2, L], FP8, name="eq8")
    xqb3 = cpool.tile([128, 2, L], BF16, name="xqb3")
    xq83 = cpool.tile([128, 2, L], FP8, name="xq83")

    xqb_ap, xq8_ap, xk_ap, xv_ap = ins["xqb"], ins["xq8"], ins["xk"], ins["xv"]

    # ================= pools =================
    es_a = ExitStack()
    kvpool = es_a.enter_context(tc.tile_pool(name="kv", bufs=2))
    rtpool = es_a.enter_context(tc.tile_pool(name="rt", bufs=4))
    prspool = es_a.enter_context(tc.tile_pool(name="prs", bufs=2))
    bpool = es_a.enter_context(tc.tile_pool(name="bnd", bufs=1))
    pq_pool = es_a.enter_context(tc.tile_pool(name="pq", bufs=2, space="PSUM"))
    ps_pool = es_a.enter_context(tc.tile_pool(name="ps", bufs=1, space="PSUM"))
    prb_pool = es_a.enter_context(tc.tile_pool(name="prb", bufs=1, space="PSUM"))
    es_ctx = ExitStack()
    bigpool = es_ctx.enter_context(tc.tile_pool(name="big", bufs=1))
    ctxpool = es_ctx.enter_context(tc.tile_pool(name="ctxp", bufs=1, space="PSUM"))
    es_kv = ExitStack()
    pkv = es_kv.enter_context(tc.tile_pool(name="pkv", bufs=2, space="PSUM"))

    ksmT = bigpool.tile([128, 2 * NU, 256], FP8, name="ksmT")
    vT = bigpool.tile([128, 2 * NU, 4, 65], FP8, name="vT")
    # ones columns for the in-gram S_k (written once, before any gram)
    nc.vector.memset(vT[:, :, :, 64:65], 1.0)

    ctx_ps = [ctxpool.tile([128, 130], F32, name=f"ctx{t}") for t in range(2)]

    def gram(u):
        # fused 2-group Gram: out rows 0:64 valid at cols 0:65 (group 2t),
        # rows 64:128 at cols 65:130 (group 2t+1); col 64|129 = S_k
        for t in range(2):
            nc.tensor.matmul(
                ctx_ps[t][:],
                ksmT[:, 2 * u : 2 * u + 2, 128 * t : 128 * t + 128],
                vT[:, 2 * u : 2 * u + 2, 2 * t : 2 * t + 2, :],
                start=(u == 0), stop=(u == NU - 1),
                perf_mode=DR,
            )

    def qwork(a):
        w = min(LW, L - a * LW)
        l0 = a * LW
        for c in range(2):
            pq = pq_pool.tile([128, w], F32, name="pq")
            nc.tensor.matmul(
                pq[:], WQ8[:, :, 128 * c : 128 * c + 128],
                xq83[:, :, l0 : l0 + w],
                start=True, stop=True, perf_mode=DR,
            )
            nc.scalar.activation(
                eq8[:, c : c + 1, l0 : l0 + w], pq[:], EXP, bias=BQL[:, c : c + 1]
            )

    rtb_tiles = {}

    def qsum(a):
        w = min(LW, L - a * LW)
        l0 = a * LW
        psS = ps_pool.tile([8, w], F32, name="psS")
        nc.tensor.matmul(
            psS[:], BONES8, eq8[:, :, l0 : l0 + w],
            start=True, stop=True, perf_mode=DR,
        )
        rt = rtpool.tile([8, w], F32, name="rt")
        rtb = rtpool.tile([8, w], BF16, name="rtb")
        nc.vector.reciprocal_approx_fast(rt[:], psS[:])
        nc.vector.tensor_copy(rtb[:], rt[:])
        rtb_tiles[a] = rtb

    def qnorm(a, c):
        # broadcast 1/S to all head partitions (PE), stage to SBUF bf16
        # (alternating ACT/DVE), then normalize on GPSIMD (sbuf-only)
        w = min(LW, L - a * LW)
        l0 = a * LW
        rtb = rtb_tiles[a] if c == 0 else rtb_tiles.pop(a)
        prb = prb_pool.tile([128, w], F32, name="prb")
        nc.tensor.matmul(prb[:], ind8[:, 128 * c : 128 * c + 128], rtb[:],
                         start=True, stop=True)
        prbs = prspool.tile([128, w], BF16, name="prbs")
        if (2 * a + c) % 2 == 1:
            nc.vector.tensor_copy(prbs[:], prb[:])
        else:
            nc.scalar.copy(prbs[:], prb[:])
        nc.gpsimd.tensor_tensor(
            eq8[:, c : c + 1, l0 : l0 + w].squeeze(),
            eq8[:, c : c + 1, l0 : l0 + w].squeeze(),
            prbs[:], op=MULT,
        )

    def qstages(t):
        if 0 <= t - 2 < NLW:
            qwork(t - 2)
        if 0 <= t - 3 < NLW:
            qsum(t - 3)
        if 0 <= t - 4 < NLW:
            qnorm(t - 4, 0)
        if 0 <= t - 5 < NLW:
            qnorm(t - 5, 1)

    # ================= pass 1: k/v proj + Gram with interleaved q =================
    xk_t = xv_t = None
    consts_loaded = False
    for a in range(NLW):
        w = min(LW, L - a * LW)
        l0 = a * LW
        if a % 2 == 0:
            # consts first (tiny, unblock the first matmul); k/v fp8 in
            # 1024-wide chunk pairs; q loads lag so they don't steal HBM
            # bandwidth from the critical early k/v stream
            if not consts_loaded:
                consts_loaded = True
                nc.sync.dma_start(c8[:], ins["c8"][:])
                nc.sync.dma_start(cf[:], ins["cf"][:])
            wd = min(2 * LW, L - l0)
            xk_t = kvpool.tile([128, 2, wd], FP8, name="xk_t")
            xv_t = kvpool.tile([128, 2, wd], FP8, name="xv_t")
            for k in range(2):
                nc.sync.dma_start(xk_t[:, k : k + 1, :], xk_ap[128 * k : 128 * (k + 1), l0 : l0 + wd])
                nc.sync.dma_start(xv_t[:, k : k + 1, :], xv_ap[128 * k : 128 * (k + 1), l0 : l0 + wd])
            for k in range(2):
                nc.sync.dma_start(
                    xq83[:, k : k + 1, l0 : l0 + wd], xq8_ap[128 * k : 128 * (k + 1), l0 : l0 + wd]
                )
            if a == 0:
                nc.sync.dma_start(wrt4[:], ins["wrt4"][:])
                nc.sync.dma_start(ind8[:], ins["ind8"][:])
            else:
                lq = (a - 2) * LW
                wq = min(2 * LW, L - lq)
                for k in range(2):
                    nc.sync.dma_start(
                        xqb3[:, k : k + 1, lq : lq + wq], xqb_ap[128 * k : 128 * (k + 1), lq : lq + wq]
                    )
        if a == NLW - 1:
            lq = (NLW - 1) * LW
            wq = L - lq
            for k in range(2):
                nc.sync.dma_start(
                    xqb3[:, k : k + 1, lq : lq + wq], xqb_ap[128 * k : 128 * (k + 1), lq : lq + wq]
                )
        off = 512 * (a % 2)
        for jj in range(w // 256):
            u = 2 * a + jj
            pk = pkv.tile([128, 512], F32, name="pkv")
            pv = pkv.tile([128, 2, 4, 64], F32, name="pkv")
            for j in range(2):
                o = off + 256 * jj + 128 * j
                nc.tensor.matmul(
                    pk[:, 256 * j : 256 * j + 256],
                    xk_t[:, :, o : o + 128], WK8,
                    start=True, stop=True, perf_mode=DR,
                )
                nc.tensor.matmul(
                    pv[:, j : j + 1, :, :],
                    xv_t[:, :, o : o + 128], WV8,
                    start=True, stop=True, perf_mode=DR,
                )
            nc.scalar.activation(
                ksmT[:, 2 * u : 2 * u + 2, :], pk[:], EXP, bias=KBIAS
            )
            nc.vector.tensor_copy(vT[:, 2 * u : 2 * u + 2, :, 0:64], pv[:])
            if u - 2 >= 0:
                gram(u - 2)
        qstages(a)

    for u in range(NU - 2, NU):
        gram(u)

    es_kv.close()  # release pk/pv banks for the boundary

    # ---------------- boundary: build at8 [kch, 2, c_out] (fp8) ----------------
    es_bnd = ExitStack()
    bpsum = es_bnd.enter_context(tc.tile_pool(name="bndp", bufs=2, space="PSUM"))
    rk = [bpool.tile([128, 1], F32, name=f"rk{t}") for t in range(2)]
    ctxs = [bpool.tile([128, 64], F32, name=f"ctxs{t}") for t in range(2)]
    for t in range(2):
        nc.vector.reciprocal(rk[t][0:64, :], ctx_ps[t][0:64, 64:65])
        nc.vector.reciprocal(rk[t][64:128, :], ctx_ps[t][64:128, 129:130])
        nc.vector.memset(ctxs[t][:], 0.0)
    for h in range(HEADS):
        t = h // 4
        gp = (h // 2) % 2          # group parity within tile
        r0 = 64 * gp + 32 * (h % 2)
        cT = 65 * gp + 32 * (h % 2)
        cs = 32 * (h % 2)
        nc.vector.tensor_scalar_mul(
            ctxs[t][r0 : r0 + 32, cs : cs + 32],
            ctx_ps[t][r0 : r0 + 32, cT : cT + 32],
            rk[t][r0 : r0 + 32, :],
        )
    ctxT_ps = [bpsum.tile([64, 128], F32, name="bnd") for t in range(2)]
    for t in range(2):
        nc.tensor.transpose(ctxT_ps[t][:], ctxs[t][:], IDENT)
    ctxT_sb = [bpool.tile([64, 128], BF16, name=f"ctxTs{t}") for t in range(2)]
    for t in range(2):
        nc.vector.tensor_copy(ctxT_sb[t][:], ctxT_ps[t][:])
    at_ps = [bpsum.tile([128, 256], F32, name="bnd") for t in range(2)]
    for g in range(4):
        t = g // 2
        half = g % 2
        nc.tensor.matmul(
            at_ps[t][64 * half : 64 * half + 64, :],
            ctxT_sb[t][:, 64 * half : 64 * half + 64],
            wrt4[:, g : g + 1, :].squeeze(),
            start=True, stop=True,
        )
    for t in range(2):
        nc.scalar.copy(at8[:, t : t + 1, :].squeeze(), at_ps[t][:])
    es_bnd.close()
    es_ctx.close()

    # ========= tail: remaining q stages interleaved with pass-2 output =========
    es_c = ExitStack()
    opool = es_c.enter_context(tc.tile_pool(name="op", bufs=3))
    po_pool = es_c.enter_context(tc.tile_pool(name="po", bufs=4, space="PSUM"))

    def pass2(a):
        wd = min(2 * LW, L - a * LW)
        ld = a * LW
        for c in range(2):
            ob = opool.tile([128, wd], BF16, name="ob")
            for half in range(0, wd, LW):
                w = min(LW, wd - half)
                l0 = ld + half
                po = po_pool.tile([128, w], F32, name="po")
                nc.tensor.matmul(
                    po[:], at8[:, :, 128 * c : 128 * c + 128],
                    eq8[:, :, l0 : l0 + w],
                    start=True, stop=True, perf_mode=DR,
                )
                nc.vector.scalar_tensor_tensor(
                    ob[:, half : half + w], po[:], BRP[:, c : c + 1],
                    xqb3[:, c : c + 1, l0 : l0 + w].squeeze(), op0=ADD, op1=ADD,
                )
            nc.sync.dma_start(out_ap[128 * c : 128 * c + 128, ld : ld + wd], ob[:])

    p2 = 0
    for t in range(NLW, NLW + 6):
        qstages(t)
        while p2 < NLW and min(p2 + 1, NLW - 1) <= t - 5:
            pass2(p2)
            p2 += 2
    es_c.close()
    es_a.close()
    es.close()


def _build_consts(Wq, bq, Wk, bk, Wv, bv, Wr, br):
    bf = ml_dtypes.bfloat16
    f8 = ml_dtypes.float8_e4m3

    def blocks(Wt):  # W [cout, cin] -> (k, 128, cout): k-subtile row blocks of W.T
        t = np.ascontiguousarray(np.asarray(Wt, np.float32).T)
        return t[0:128, :], t[128:256, :]

    ch = np.arange(256)
    bones_full = (ch[:, None] // 32 == np.arange(8)[None, :]).astype(np.float32)

    # c8: [128, 2, 776] flattened to [128, 1552]; inner = wk|wv|wq|bones
    c8 = np.zeros((128, 2, C8_INNER), np.float32)
    for k in range(2):
        c8[:, k, 0:256] = blocks(Wk)[k]
        c8[:, k, 256:512] = blocks(Wv)[k]
        c8[:, k, 512:768] = blocks(Wq)[k]
        c8[:, k, 768:776] = bones_full[128 * k : 128 * (k + 1), :]
    c8 = c8.reshape(128, 2 * C8_INNER).astype(f8)

    ind8 = np.ascontiguousarray(bones_full.T).astype(bf)          # [8, 256]

    WrT = np.ascontiguousarray(np.asarray(Wr, np.float32).T)      # [vch, cout]
    wrt4 = np.concatenate([WrT[64 * b : 64 * (b + 1), :] for b in range(4)],
                          axis=1).astype(bf)                      # [64, 1024]

    brp = np.asarray(br, np.float32) + np.asarray(Wr, np.float32) @ np.asarray(bv, np.float32)

    def two(v):
        return np.stack([v[0:128], v[128:256]], axis=1).astype(np.float32)

    cf = np.concatenate(
        [two(np.asarray(bq, np.float32) - LN4), two(brp),
         np.full((128, 1), -LN4, np.float32), np.eye(128, dtype=np.float32)],
        axis=1,
    ).astype(np.float32)
    assert cf.shape == (128, CF_COLS), cf.shape
    return {"c8": c8, "ind8": ind8, "wrt4": wrt4, "cf": cf}


_NC = None


def _build():
    nc = bacc.Bacc("TRN2", target_bir_lowering=False)
    ins = {}
    ins["xqb"] = nc.dram_tensor("xqb", [CIN, L], BF16, kind="ExternalInput").ap()
    ins["xq8"] = nc.dram_tensor("xq8", [CIN, L], FP8, kind="ExternalInput").ap()
    ins["xk"] = nc.dram_tensor("xk", [CIN, L], FP8, kind="ExternalInput").ap()
    ins["xv"] = nc.dram_tensor("xv", [CIN, L], FP8, kind="ExternalInput").ap()
    ins["c8"] = nc.dram_tensor("c8", [128, 2 * C8_INNER], FP8, kind="ExternalInput").ap()
    ins["ind8"] = nc.dram_tensor("ind8", [8, 256], BF16, kind="ExternalInput").ap()
    ins["wrt4"] = nc.dram_tensor("wrt4", [64, 1024], BF16, kind="ExternalInput").ap()
    ins["cf"] = nc.dram_tensor("cf", [128, CF_COLS], F32, kind="ExternalInput").ap()
    out_ap = nc.dram_tensor("out", [CIN, L], BF16, kind="ExternalOutput").ap()
    with tile.TileContext(nc) as tc:
        _emit(tc, ins, out_ap)
    nc.compile()
    return nc


def get_nc():
    global _NC
    if _NC is None:
        _NC = _build()
    return _NC


def make_in_maps(inputs):
    bf = ml_dtypes.bfloat16
    f8 = ml_dtypes.float8_e4m3
    consts = _build_consts(
        inputs["Wq"], inputs["bq"], inputs["Wk"], inputs["bk"],
        inputs["Wv"], inputs["bv"], inputs["Wr"], inputs["br"],
    )
    qf = np.asarray(inputs["query_feature"], np.float32).reshape(N, CIN, L)
    kf = np.asarray(inputs["key_feature"], np.float32).reshape(N, CIN, L)
    vf = np.asarray(inputs["value_feature"], np.float32).reshape(N, CIN, L)
    qfb = qf.astype(bf)
    qf8 = qf.astype(f8)
    kf8 = kf.astype(f8)
    vf8 = vf.astype(f8)
    return [
        {"xqb": np.ascontiguousarray(qfb[i]),
         "xq8": np.ascontiguousarray(qf8[i]),
         "xk": np.ascontiguousarray(kf8[i]),
         "xv": np.ascontiguousarray(vf8[i]),
         **consts}
        for i in range(N_CORES)
    ]


def kernel(query_feature, key_feature, value_feature,
           Wq, bq, Wk, bk, Wv, bv, Wr, br):
    nc = get_nc()
    in_maps = make_in_maps(dict(
        query_feature=query_feature, key_feature=key_feature,
        value_feature=value_feature, Wq=Wq, bq=bq, Wk=Wk, bk=bk,
        Wv=Wv, bv=bv, Wr=Wr, br=br,
    ))
    res = run_bass_kernel_spmd(nc, in_maps, core_ids=list(range(N_CORES)))
    out = np.stack([
        np.asarray(res.results[i]["out"]).astype(np.float32) for i in range(N_CORES)
    ])
    return out.reshape(N, CIN, H_IMG, W_IMG)


# revision 18
# speedup vs baseline: 1.0084x; 1.0084x over previous
"""CrossEfficientAttention on 8 Trainium2 NeuronCores — fp8 DoubleRow version.

Batch-parallel sharding: n=8 batch items, one per core (no collectives).

Per-core math (item x_q, x_k, x_v : [256, 6400]):
    q  = Wq x_q + bq ; k = Wk x_k (bk cancels over the l-softmax) ; v = Wv x_v
    k_sm = softmax_l(k); q_sm = softmax_ch/head(q)
    ctx  = k_sm @ v^T (per head); out = Wr @ (ctx^T @ q_sm) + br' + x_q
  with br' = br + Wr @ bv (the bv term reduces to a constant because
  sum_head q_sm == 1), so bv never appears on the device.

Numerics: the attention term is a few % of the output (the residual
dominates), so the whole attention path runs in fp8e4m3 on the PE with
DoubleRow perf mode (256-deep contraction in one matmul, ~2x bf16 rate);
exp() outputs are scaled by 1/4 (bias -ln4) to stay inside fp8 range —
the scale cancels exactly in both softmax normalizations.  The residual
is bf16; output ships bf16.

Structure (one fused streaming loop + tiny boundary + output pass):
  Pass 1 per 256-l unit: k/v projected via one DoubleRow matmul per
    128-l block into [l, ch] psum, exp(k) on ACT -> fp8 ksmT, v copied
    on DVE -> fp8 vT (4-D layout with a ones column per 64-ch group);
    Gram ctx accumulates via 2 fused [128,130] DoubleRow matmuls per
    unit — the ones column yields S_k for free (cross-group halves of
    each 130-wide output are garbage and never read).  Interleaved q
    pipeline: DoubleRow q-projection, exp on ACT, head-sums via one
    DoubleRow matmul with a 0/1 block matrix, fast reciprocal, PE
    broadcast, normalize in place on GPSIMD.
  Boundary: 1/S_k scale of the per-head diag blocks, 2 PE transposes,
    at = ctxn^T . Wr^T via 4 small matmuls, cast to fp8.
  Pass 2: po = at^T . q_sm (one DoubleRow matmul per 512 cols), then
    out = po + br' + x_q via one scalar_tensor_tensor -> bf16 -> DMA.
"""

from contextlib import ExitStack

import ml_dtypes
import numpy as np

import concourse.bacc as bacc
import concourse.bass as bass
import concourse.tile as tile
from concourse import mybir
from concourse.bass_utils import run_bass_kernel_spmd

F32 = mybir.dt.float32
BF16 = mybir.dt.bfloat16
FP8 = mybir.dt.float8e4
EXP = mybir.ActivationFunctionType.Exp
DR = mybir.MatmulPerfMode.DoubleRow
MULT = mybir.AluOpType.mult
ADD = mybir.AluOpType.add

N_CORES = 8
N, CIN, H_IMG, W_IMG = 8, 256, 80, 80
L = H_IMG * W_IMG            # 6400
HEADS = 8
LW = 512                     # streaming l tile width
NLW = (L + LW - 1) // LW     # 13 (12x512 + 1x256)
NU = L // 256                # 25 gram units of 256 l

LN4 = float(np.log(4.0))

# c8 fp8 const tile [128, 2, 776]: per k-subtile: wk(256)|wv(256)|wq(256)|bones(8)
C8_INNER = 784
CF_COLS = 133                # f32: bql [128,2] | brp [128,2] | kbias | ident


def _emit(tc: tile.TileContext, ins: dict, out_ap: bass.AP):
    nc = tc.nc
    es = ExitStack()

    # ---------------- persistent consts (4 DMAs) ----------------
    cpool = es.enter_context(tc.tile_pool(name="consts", bufs=1))
    c8 = cpool.tile([128, 2, C8_INNER], FP8, name="c8")
    ind8 = cpool.tile([8, 256], BF16, name="ind8")
    wrt4 = cpool.tile([64, 4, 256], BF16, name="wrt4")
    cf = cpool.tile([128, CF_COLS], F32, name="cf")
    WK8 = c8[:, :, 0:256]
    WV8 = c8[:, :, 256:512]
    WQ8 = c8[:, :, 512:768]
    BONES8 = c8[:, :, 768:776]
    BQL = cf[:, 0:2]
    BRP = cf[:, 2:4]
    KBIAS = cf[:, 4:5]
    IDENT = cf[:, 5:133]

    at8 = cpool.tile([128, 2, 256], FP8, name="at8")
    eq8 = cpool.tile([128, 2, L], FP8, name="eq8")
    xqb3 = cpool.tile([128, 2, L], BF16, name="xqb3")
    xq83 = cpool.tile([128, 2, L], FP8, name="xq83")

    xqb_ap, xq8_ap, xk_ap, xv_ap = ins["xqb"], ins["xq8"], ins["xk"], ins["xv"]

    # ================= pools =================
    es_a = ExitStack()
    kvpool = es_a.enter_context(tc.tile_pool(name="kv", bufs=2))
    rtpool = es_a.enter_context(tc.tile_pool(name="rt", bufs=4))
    prspool = es_a.enter_context(tc.tile_pool(name="prs", bufs=3))
    bpool = es_a.enter_context(tc.tile_pool(name="bnd", bufs=1))
    pq_pool = es_a.enter_context(tc.tile_pool(name="pq", bufs=2, space="PSUM"))
    ps_pool = es_a.enter_context(tc.tile_pool(name="ps", bufs=1, space="PSUM"))
    prb_pool = es_a.enter_context(tc.tile_pool(name="prb", bufs=1, space="PSUM"))
    es_ctx = ExitStack()
    bigpool = es_ctx.enter_context(tc.tile_pool(name="big", bufs=1))
    ctxpool = es_ctx.enter_context(tc.tile_pool(name="ctxp", bufs=1, space="PSUM"))
    es_kv = ExitStack()
    pkv = es_kv.enter_context(tc.tile_pool(name="pkv", bufs=2, space="PSUM"))

    ksmT = bigpool.tile([128, 2 * NU, 256], FP8, name="ksmT")
    vT = bigpool.tile([128, 2 * NU, 4, 65], FP8, name="vT")
    # ones columns for the in-gram S_k (written once, before any gram)
    nc.vector.memset(vT[:, :, :, 64:65], 1.0)

    ctx_ps = [ctxpool.tile([128, 130], F32, name=f"ctx{t}") for t in range(2)]

    def gram(u):
        # fused 2-group Gram: out rows 0:64 valid at cols 0:65 (group 2t),
        # rows 64:128 at cols 65:130 (group 2t+1); col 64|129 = S_k
        for t in range(2):
            nc.tensor.matmul(
                ctx_ps[t][:],
                ksmT[:, 2 * u : 2 * u + 2, 128 * t : 128 * t + 128],
                vT[:, 2 * u : 2 * u + 2, 2 * t : 2 * t + 2, :],
                start=(u == 0), stop=(u == NU - 1),
                perf_mode=DR,
            )

    def qwork(a):
        w = min(LW, L - a * LW)
        l0 = a * LW
        for c in range(2):
            pq = pq_pool.tile([128, w], F32, name="pq")
            nc.tensor.matmul(
                pq[:], WQ8[:, :, 128 * c : 128 * c + 128],
                xq83[:, :, l0 : l0 + w],
                start=True, stop=True, perf_mode=DR,
            )
            nc.scalar.activation(
                eq8[:, c : c + 1, l0 : l0 + w], pq[:], EXP, bias=BQL[:, c : c + 1]
            )

    rtb_tiles = {}

    def qsum(a):
        w = min(LW, L - a * LW)
        l0 = a * LW
        psS = ps_pool.tile([8, w], F32, name="psS")
        nc.tensor.matmul(
            psS[:], BONES8, eq8[:, :, l0 : l0 + w],
            start=True, stop=True, perf_mode=DR,
        )
        rt = rtpool.tile([8, w], F32, name="rt")
        rtb = rtpool.tile([8, w], BF16, name="rtb")
        nc.vector.reciprocal_approx_fast(rt[:], psS[:])
        nc.vector.tensor_copy(rtb[:], rt[:])
        rtb_tiles[a] = rtb

    def qnorm(a, c):
        # broadcast 1/S to all head partitions (PE), stage to SBUF bf16
        # (alternating ACT/DVE), then normalize on GPSIMD (sbuf-only)
        w = min(LW, L - a * LW)
        l0 = a * LW
        rtb = rtb_tiles[a] if c == 0 else rtb_tiles.pop(a)
        prb = prb_pool.tile([128, w], F32, name="prb")
        nc.tensor.matmul(prb[:], ind8[:, 128 * c : 128 * c + 128], rtb[:],
                         start=True, stop=True)
        prbs = prspool.tile([128, w], BF16, name="prbs")
        if (2 * a + c) % 2 == 1 and a < 8:
            nc.vector.tensor_copy(prbs[:], prb[:])
        else:
            nc.scalar.copy(prbs[:], prb[:])
        nc.gpsimd.tensor_tensor(
            eq8[:, c : c + 1, l0 : l0 + w].squeeze(),
            eq8[:, c : c + 1, l0 : l0 + w].squeeze(),
            prbs[:], op=MULT,
        )

    def qstages(t):
        if 0 <= t - 2 < NLW:
            qwork(t - 2)
        if 0 <= t - 3 < NLW:
            qsum(t - 3)
        if 0 <= t - 4 < NLW:
            qnorm(t - 4, 0)
        if 0 <= t - 5 < NLW:
            qnorm(t - 5, 1)

    # ================= pass 1: k/v proj + Gram with interleaved q =================
    xk_t = xv_t = None
    consts_loaded = False
    for a in range(NLW):
        w = min(LW, L - a * LW)
        l0 = a * LW
        if a % 2 == 0:
            # consts first (tiny, unblock the first matmul); k/v fp8 in
            # 1024-wide chunk pairs; q loads lag so they don't steal HBM
            # bandwidth from the critical early k/v stream
            if not consts_loaded:
                consts_loaded = True
                nc.sync.dma_start(c8[:], ins["c8"][:])
                nc.sync.dma_start(cf[:], ins["cf"][:])
            wd = min(2 * LW, L - l0)
            xk_t = kvpool.tile([128, 2, wd], FP8, name="xk_t")
            xv_t = kvpool.tile([128, 2, wd], FP8, name="xv_t")
            for k in range(2):
                nc.sync.dma_start(xk_t[:, k : k + 1, :], xk_ap[128 * k : 128 * (k + 1), l0 : l0 + wd])
                nc.sync.dma_start(xv_t[:, k : k + 1, :], xv_ap[128 * k : 128 * (k + 1), l0 : l0 + wd])
            for k in range(2):
                nc.sync.dma_start(
                    xq83[:, k : k + 1, l0 : l0 + wd], xq8_ap[128 * k : 128 * (k + 1), l0 : l0 + wd]
                )
            if a == 0:
                nc.sync.dma_start(wrt4[:], ins["wrt4"][:])
                nc.sync.dma_start(ind8[:], ins["ind8"][:])
            else:
                lq = (a - 2) * LW
                wq = min(2 * LW, L - lq)
                for k in range(2):
                    nc.sync.dma_start(
                        xqb3[:, k : k + 1, lq : lq + wq], xqb_ap[128 * k : 128 * (k + 1), lq : lq + wq]
                    )
        if a == NLW - 1:
            lq = (NLW - 1) * LW
            wq = L - lq
            for k in range(2):
                nc.sync.dma_start(
                    xqb3[:, k : k + 1, lq : lq + wq], xqb_ap[128 * k : 128 * (k + 1), lq : lq + wq]
                )
        off = 512 * (a % 2)
        for jj in range(w // 256):
            u = 2 * a + jj
            pk = pkv.tile([128, 512], F32, name="pkv")
            pv = pkv.tile([128, 2, 4, 64], F32, name="pkv")
            for j in range(2):
                o = off + 256 * jj + 128 * j
                nc.tensor.matmul(
                    pk[:, 256 * j : 256 * j + 256],
                    xk_t[:, :, o : o + 128], WK8,
                    start=True, stop=True, perf_mode=DR,
                )
                nc.tensor.matmul(
                    pv[:, j : j + 1, :, :],
                    xv_t[:, :, o : o + 128], WV8,
                    start=True, stop=True, perf_mode=DR,
                )
            nc.scalar.activation(
                ksmT[:, 2 * u : 2 * u + 2, :], pk[:], EXP, bias=KBIAS
            )
            nc.vector.tensor_copy(vT[:, 2 * u : 2 * u + 2, :, 0:64], pv[:])
            if u - 2 >= 0:
                gram(u - 2)
        qstages(a)

    for u in range(NU - 2, NU):
        gram(u)

    es_kv.close()  # release pk/pv banks for the boundary

    # ---------------- boundary: build at8 [kch, 2, c_out] (fp8) ----------------
    es_bnd = ExitStack()
    bpsum = es_bnd.enter_context(tc.tile_pool(name="bndp", bufs=2, space="PSUM"))
    rk = [bpool.tile([128, 1], F32, name=f"rk{t}") for t in range(2)]
    ctxs = [bpool.tile([128, 64], F32, name=f"ctxs{t}") for t in range(2)]
    for t in range(2):
        nc.vector.reciprocal(rk[t][0:64, :], ctx_ps[t][0:64, 64:65])
        nc.vector.reciprocal(rk[t][64:128, :], ctx_ps[t][64:128, 129:130])
        nc.vector.memset(ctxs[t][:], 0.0)
    for h in range(HEADS):
        t = h // 4
        gp = (h // 2) % 2          # group parity within tile
        r0 = 64 * gp + 32 * (h % 2)
        cT = 65 * gp + 32 * (h % 2)
        cs = 32 * (h % 2)
        nc.vector.tensor_scalar_mul(
            ctxs[t][r0 : r0 + 32, cs : cs + 32],
            ctx_ps[t][r0 : r0 + 32, cT : cT + 32],
            rk[t][r0 : r0 + 32, :],
        )
    ctxT_ps = [bpsum.tile([64, 128], F32, name="bnd") for t in range(2)]
    for t in range(2):
        nc.tensor.transpose(ctxT_ps[t][:], ctxs[t][:], IDENT)
    ctxT_sb = [bpool.tile([64, 128], BF16, name=f"ctxTs{t}") for t in range(2)]
    for t in range(2):
        nc.vector.tensor_copy(ctxT_sb[t][:], ctxT_ps[t][:])
    at_ps = [bpsum.tile([128, 256], F32, name="bnd") for t in range(2)]
    for g in range(4):
        t = g // 2
        half = g % 2
        nc.tensor.matmul(
            at_ps[t][64 * half : 64 * half + 64, :],
            ctxT_sb[t][:, 64 * half : 64 * half + 64],
            wrt4[:, g : g + 1, :].squeeze(),
            start=True, stop=True,
        )
    for t in range(2):
        nc.scalar.copy(at8[:, t : t + 1, :].squeeze(), at_ps[t][:])
    es_bnd.close()
    es_ctx.close()

    # ========= tail: remaining q stages interleaved with pass-2 output =========
    es_c = ExitStack()
    opool = es_c.enter_context(tc.tile_pool(name="op", bufs=3))
    po_pool = es_c.enter_context(tc.tile_pool(name="po", bufs=4, space="PSUM"))

    COPY = mybir.ActivationFunctionType.Copy

    def pass2(a):
        wd = min(2 * LW, L - a * LW)
        ld = a * LW
        for c in range(2):
            ob = opool.tile([128, wd], BF16, name="ob")
            for hi, half in enumerate(range(0, wd, LW)):
                w = min(LW, wd - half)
                l0 = ld + half
                po = po_pool.tile([128, w], F32, name="po")
                nc.tensor.matmul(
                    po[:], at8[:, :, 128 * c : 128 * c + 128],
                    eq8[:, :, l0 : l0 + w],
                    start=True, stop=True, perf_mode=DR,
                )
                if (2 * c + hi) % 2 == 0:
                    # DVE path: psum + (x_q + br') in one pass
                    nc.vector.tensor_tensor(
                        ob[:, half : half + w], po[:],
                        xqb3[:, c : c + 1, l0 : l0 + w].squeeze(), op=ADD,
                    )
                else:
                    # ACT evicts psum, idle GPSIMD adds the residual
                    tmp = prspool.tile([128, w], BF16, name="tmp")
                    nc.scalar.copy(tmp[:], po[:])
                    nc.gpsimd.tensor_tensor(
                        ob[:, half : half + w], tmp[:],
                        xqb3[:, c : c + 1, l0 : l0 + w].squeeze(), op=ADD,
                    )
            nc.sync.dma_start(out_ap[128 * c : 128 * c + 128, ld : ld + wd], ob[:])

    p2 = 0
    for t in range(NLW, NLW + 6):
        qstages(t)
        while p2 < NLW and min(p2 + 1, NLW - 1) <= t - 5:
            pass2(p2)
            p2 += 2
    es_c.close()
    es_a.close()
    es.close()


def _build_consts(Wq, bq, Wk, bk, Wv, bv, Wr, br):
    bf = ml_dtypes.bfloat16
    f8 = ml_dtypes.float8_e4m3

    def blocks(Wt):  # W [cout, cin] -> (k, 128, cout): k-subtile row blocks of W.T
        t = np.ascontiguousarray(np.asarray(Wt, np.float32).T)
        return t[0:128, :], t[128:256, :]

    ch = np.arange(256)
    bones_full = (ch[:, None] // 32 == np.arange(8)[None, :]).astype(np.float32)

    # c8: [128, 2, 776] flattened to [128, 1552]; inner = wk|wv|wq|bones
    c8 = np.zeros((128, 2, C8_INNER), np.float32)
    for k in range(2):
        c8[:, k, 0:256] = blocks(Wk)[k]
        c8[:, k, 256:512] = blocks(Wv)[k]
        c8[:, k, 512:768] = blocks(Wq)[k]
        c8[:, k, 768:776] = bones_full[128 * k : 128 * (k + 1), :]
    c8 = c8.reshape(128, 2 * C8_INNER).astype(f8)

    ind8 = np.ascontiguousarray(bones_full.T).astype(bf)          # [8, 256]

    WrT = np.ascontiguousarray(np.asarray(Wr, np.float32).T)      # [vch, cout]
    wrt4 = np.concatenate([WrT[64 * b : 64 * (b + 1), :] for b in range(4)],
                          axis=1).astype(bf)                      # [64, 1024]

    brp = np.asarray(br, np.float32) + np.asarray(Wr, np.float32) @ np.asarray(bv, np.float32)

    def two(v):
        return np.stack([v[0:128], v[128:256]], axis=1).astype(np.float32)

    cf = np.concatenate(
        [two(np.asarray(bq, np.float32) - LN4), two(brp),
         np.full((128, 1), -LN4, np.float32), np.eye(128, dtype=np.float32)],
        axis=1,
    ).astype(np.float32)
    assert cf.shape == (128, CF_COLS), cf.shape
    return {"c8": c8, "ind8": ind8, "wrt4": wrt4, "cf": cf}


_NC = None


def _build():
    nc = bacc.Bacc("TRN2", target_bir_lowering=False)
    ins = {}
    ins["xqb"] = nc.dram_tensor("xqb", [CIN, L], BF16, kind="ExternalInput").ap()
    ins["xq8"] = nc.dram_tensor("xq8", [CIN, L], FP8, kind="ExternalInput").ap()
    ins["xk"] = nc.dram_tensor("xk", [CIN, L], FP8, kind="ExternalInput").ap()
    ins["xv"] = nc.dram_tensor("xv", [CIN, L], FP8, kind="ExternalInput").ap()
    ins["c8"] = nc.dram_tensor("c8", [128, 2 * C8_INNER], FP8, kind="ExternalInput").ap()
    ins["ind8"] = nc.dram_tensor("ind8", [8, 256], BF16, kind="ExternalInput").ap()
    ins["wrt4"] = nc.dram_tensor("wrt4", [64, 1024], BF16, kind="ExternalInput").ap()
    ins["cf"] = nc.dram_tensor("cf", [128, CF_COLS], F32, kind="ExternalInput").ap()
    out_ap = nc.dram_tensor("out", [CIN, L], BF16, kind="ExternalOutput").ap()
    with tile.TileContext(nc) as tc:
        _emit(tc, ins, out_ap)
    nc.compile()
    return nc


def get_nc():
    global _NC
    if _NC is None:
        _NC = _build()
    return _NC


def make_in_maps(inputs):
    bf = ml_dtypes.bfloat16
    f8 = ml_dtypes.float8_e4m3
    consts = _build_consts(
        inputs["Wq"], inputs["bq"], inputs["Wk"], inputs["bk"],
        inputs["Wv"], inputs["bv"], inputs["Wr"], inputs["br"],
    )
    qf = np.asarray(inputs["query_feature"], np.float32).reshape(N, CIN, L)
    kf = np.asarray(inputs["key_feature"], np.float32).reshape(N, CIN, L)
    vf = np.asarray(inputs["value_feature"], np.float32).reshape(N, CIN, L)
    brp = (np.asarray(inputs["br"], np.float32)
           + np.asarray(inputs["Wr"], np.float32) @ np.asarray(inputs["bv"], np.float32))
    qfb = (qf + brp[None, :, None]).astype(bf)
    qf8 = qf.astype(f8)
    kf8 = kf.astype(f8)
    vf8 = vf.astype(f8)
    return [
        {"xqb": np.ascontiguousarray(qfb[i]),
         "xq8": np.ascontiguousarray(qf8[i]),
         "xk": np.ascontiguousarray(kf8[i]),
         "xv": np.ascontiguousarray(vf8[i]),
         **consts}
        for i in range(N_CORES)
    ]


def kernel(query_feature, key_feature, value_feature,
           Wq, bq, Wk, bk, Wv, bv, Wr, br):
    nc = get_nc()
    in_maps = make_in_maps(dict(
        query_feature=query_feature, key_feature=key_feature,
        value_feature=value_feature, Wq=Wq, bq=bq, Wk=Wk, bk=bk,
        Wv=Wv, bv=bv, Wr=Wr, br=br,
    ))
    res = run_bass_kernel_spmd(nc, in_maps, core_ids=list(range(N_CORES)))
    out = np.stack([
        np.asarray(res.results[i]["out"]).astype(np.float32) for i in range(N_CORES)
    ])
    return out.reshape(N, CIN, H_IMG, W_IMG)
